# revision 1
# baseline (speedup 1.0000x reference)
"""TRN2 Bass kernel for nn_Attention_86260123173325.

Single-head attention over N=4096 tokens, feature dim HW=4096:
  q, k, v = x[:,0], x[:,1], x[:,2] reshaped to [4096, 4096]
  out = softmax(0.5 * q @ k.T) @ v

Sharding: q rows split across 8 cores (512 rows each); k, v replicated.
Host-side marshaling pre-transposes q and k into the PE-ready
contraction-major layouts (the PE reduces along the partition dim, so both
phase-1 operands need the feature dim on partitions); this removes ~1150
on-chip PE transposes + their PSUM->SBUF copies per core.

Per-core algorithm (matmuls in f32r = TF32-like, 4x the fp32 PE rate;
measured ~12.4-bit mantissa on HW, which keeps softmax argmax flips to
near-tied rows only => ~4e-3 absmax-relative output error):
  - Phase 1, per 128-row k block j:
    R^T[j,:] = k_j @ q^T via 32 accumulated f32r matmuls (R = raw q.k dots,
    dp = 0.5*R). Keep R^T in SBUF (fp32), and accumulate a row statistic
    W_i = sum_j exp(0.2*dp_ij - 40) via exp on ACT + ones^T @ E_t matmuls
    ([2,512] PSUM row, full-rate N=512). The -40 bias keeps W far below
    ~2^64 where the HW exp/f32r/ln chain was observed to break.
  - shift_i = 5*(ln(W_i) + 40) = LSE_{t=0.2}/t >= rowmax_i. Any per-row
    shift cancels in the final normalization, so exp(dp - shift) is an
    exact softmax numerator, can never overflow (dp - shift <= 0), and
    loses only terms below fp32 output resolution. Works for any data
    with |dp| <~ 420.
  - Pass 2: eT = exp(0.5*R - shift), f32r.
  - rowsum via E^T.T @ ones ([128,2] psum, f32r needs even free sizes)
  - Phase 2: O = (E @ v) * (1/rowsum), accumulating over j blocks in PSUM.
"""
import sys

sys.path.insert(0, "/opt/trn_rl_repo")

import numpy as np

import concourse.bass as bass
import concourse.tile as tile
from concourse import bacc, mybir
from concourse.bass_utils import run_bass_kernel_spmd

F32 = mybir.dt.float32
F32R = mybir.dt.float32r
EXP = mybir.ActivationFunctionType.Exp
LN = mybir.ActivationFunctionType.Ln

N_CORES = 8
N = 4096          # tokens (keys)
D = 4096          # feature dim (H*W)
M = N // N_CORES  # q rows per core = 512
NJ = N // 128     # 32 key blocks
ND = D // 128     # 32 feature blocks
NI = M // 128     # 4 q-row blocks per core
NDT = D // 512    # 8 output column tiles
T_STAT = 0.2      # stage-1 temperature: exp(t*dp - 40) = exp(0.1*R - 40)
STAT_BIAS = 40.0


def _build_nc():
    nc = bacc.Bacc(None, target_bir_lowering=False, debug=False)

    # qT[p, db, i] = q[i, db*128+p]; kT[jb, p, db, jj] = k[jb*128+jj, db*128+p]
    qT_dram = nc.dram_tensor("qT", [128, ND, M], F32R, kind="ExternalInput")
    kT_dram = nc.dram_tensor("kT", [NJ, 128, ND, 128], F32R, kind="ExternalInput")
    v_dram = nc.dram_tensor("v", [N, D], F32R, kind="ExternalInput")
    o_dram = nc.dram_tensor("o", [M, D], F32, kind="ExternalOutput")

    with tile.TileContext(nc) as tc:
        with tc.tile_pool(name="persist", bufs=1) as persist:
            # R^T storage, [j-within-block, j-block, i] (fp32, exact scores)
            s_sb = persist.tile([128, NJ, M], F32)

            # all-ones [128,128]: W-stat lhsT (output lands broadcast on all
            # 128 partitions); [:, 0:2] slices serve the rowsum matmuls
            # (f32r requires even free sizes on all matmul operands)
            ones_f = persist.tile([128, 128], F32, tag="ones_f")
            nc.vector.memset(ones_f[:], 1.0)
            ones_r = persist.tile([128, 128], F32R, tag="ones_r")
            nc.vector.tensor_copy(ones_r[:], ones_f[:])

            zero_b = persist.tile([128, 1], F32, tag="zero_b")
            nc.vector.memset(zero_b[:], 0.0)

            # stage-1 exp bias: keeps W = sum exp(0.2*dp - 40) well under
            # ~2^64, where the HW exp/f32r-matmul/ln chain was observed to
            # produce junk (rows with rowmax>232 went NaN without it)
            stat_b = persist.tile([128, 1], F32, tag="stat_b")
            nc.vector.memset(stat_b[:], -STAT_BIAS)

            sh2_bc = persist.tile([128, M], F32, tag="sh2_bc")
            rsum = persist.tile([128, NI], F32, tag="rsum")
            rinv = persist.tile([128, NI], F32, tag="rinv")

            # ---------------- phase 1: R^T blocks + W stats ----------------
            with (
                tc.tile_pool(name="qT", bufs=1) as qTpool,
                tc.tile_pool(name="kT", bufs=3) as kTpool,
                tc.tile_pool(name="ett", bufs=2) as etpool,
                tc.tile_pool(name="psS", bufs=2, space="PSUM") as psS,
                tc.tile_pool(name="psW", bufs=1, space="PSUM") as psWp,
            ):
                # 8 chunk tiles -> fine-grained deps; matmuls start after
                # the first 1MB chunk instead of the full 8.4MB load
                qT_parts = []
                for b in range(8):
                    qp = qTpool.tile([128, ND // 8, M], F32R, tag=f"qT{b}",
                                     name=f"qT{b}")
                    nc.scalar.dma_start(
                        out=qp[:], in_=qT_dram[:, b * (ND // 8):(b + 1) * (ND // 8), :]
                    )
                    qT_parts.append(qp)

                def qT_slice(dblk):
                    return qT_parts[dblk // (ND // 8)][:, dblk % (ND // 8), :]

                psW = psWp.tile([128, M], F32)
                for j in range(NJ):
                    kT = kTpool.tile([128, ND, 128], F32R, tag="kT",
                                     name=f"kT{j}")
                    nc.sync.dma_start(out=kT[:], in_=kT_dram[j])
                    ps = psS.tile([128, M], F32, tag="S", name=f"ps{j}")
                    for dblk in range(ND):
                        nc.tensor.matmul(
                            ps[:],
                            kT[:, dblk, :],
                            qT_slice(dblk),
                            start=(dblk == 0),
                            stop=(dblk == ND - 1),
                        )
                    # stash raw scores R^T (fp32)
                    nc.vector.tensor_copy(s_sb[:, j, :], ps[:])
                    # W stat: exp(0.1*R - 40) then ones^T @ E_t -> psW [2, M]
                    ett = etpool.tile([128, M], F32R, tag="ett", name=f"et{j}")
                    nc.scalar.activation(
                        out=ett[:], in_=ps[:], func=EXP,
                        bias=stat_b[:], scale=0.5 * T_STAT,
                    )
                    nc.tensor.matmul(
                        psW[:],
                        ones_r[:],
                        ett[:],
                        start=(j == 0),
                        stop=(j == NJ - 1),
                        skip_group_check=True,
                    )

                # sh2 = 2*shift = (2/t)*(lnW' + 40); psW rows are identical
                # (all-ones lhsT), so this lands already broadcast
                w_ln = persist.tile([128, M], F32, tag="w_ln")
                nc.scalar.activation(
                    out=w_ln[:], in_=psW[:], func=LN,
                    bias=zero_b[:], scale=1.0,
                )
                nc.vector.tensor_scalar(
                    sh2_bc[:], w_ln[:], 2.0 / T_STAT,
                    STAT_BIAS * 2.0 / T_STAT,
                    mybir.AluOpType.mult, mybir.AluOpType.add,
                )

            # phase-2 pools opened early: pre-issue the first v loads so
            # the PE has phase-2 work ready right after the exp pass starts
            _vstack = tc.tile_pool(name="vsrc", bufs=6)
            _ostack = tc.tile_pool(name="osb", bufs=6)
            _pstack = tc.tile_pool(name="psO", bufs=8, space="PSUM")
            vpool = _vstack.__enter__()
            opool = _ostack.__enter__()
            psO = _pstack.__enter__()
            v_pre = {}
            for jpre in range(6):
                vsb = vpool.tile([128, 512], F32R, tag="v", name=f"vp{jpre}")
                nc.sync.dma_start(
                    out=vsb[:], in_=v_dram[jpre * 128:(jpre + 1) * 128, 0:512]
                )
                v_pre[(0, jpre)] = vsb

            # ---------------- pass 2: eT = exp(0.5*R - shift) ------
            # (separate f32r tensor: the BIR verifier requires f32r matmul
            # inputs to be produced rounded)
            eTstack = tc.tile_pool(name="eTp", bufs=1)
            eTpool = eTstack.__enter__()
            eT_t = eTpool.tile([128, NJ, M], F32R, name="eT_t")
            with tc.tile_pool(name="tmp", bufs=3) as tmpool:
                for j in range(NJ):
                    tmp = tmpool.tile([128, M], F32, tag="tmp", name=f"tmp{j}")
                    nc.vector.tensor_sub(tmp[:], s_sb[:, j, :], sh2_bc[:])
                    nc.scalar.activation(
                        out=eT_t[:, j, :], in_=tmp[:],
                        func=EXP, bias=zero_b[:], scale=0.5,
                    )

            def eT(j, i0, i1):
                return eT_t[:, j, i0:i1]

            # ---------------- rowsums (share the 8-bank psO pool) ----------
            if True:
                for ib in range(NI):
                    pr = psO.tile([128, 2], F32, tag="o", name=f"pr{ib}")
                    for j in range(NJ):
                        nc.tensor.matmul(
                            pr[:],
                            eT(j, ib * 128, (ib + 1) * 128),
                            ones_r[:, 0:2],
                            start=(j == 0),
                            stop=(j == NJ - 1),
                        )
                    nc.vector.tensor_copy(rsum[:, ib:ib + 1], pr[:, 0:1])
                nc.vector.reciprocal(rinv[:], rsum[:])

            # ---------------- phase 2: O = (E @ v) * rinv ----------------
            if True:
                for dt in range(NDT):
                    pos = [
                        psO.tile([128, 512], F32, tag="o", name=f"po{dt}_{ib}")
                        for ib in range(NI)
                    ]
                    for j in range(NJ):
                        vsb = v_pre.pop((dt, j), None)
                        if vsb is None:
                            vsb = vpool.tile([128, 512], F32R, tag="v",
                                             name=f"v{dt}_{j}")
                            nc.sync.dma_start(
                                out=vsb[:],
                                in_=v_dram[j * 128:(j + 1) * 128,
                                           dt * 512:(dt + 1) * 512],
                            )
                        for ib in range(NI):
                            nc.tensor.matmul(
                                pos[ib][:],
                                eT(j, ib * 128, (ib + 1) * 128),
                                vsb[:],
                                start=(j == 0),
                                stop=(j == NJ - 1),
                            )
                    for ib in range(NI):
                        osb = opool.tile([128, 512], F32, tag="osb",
                                         name=f"ob{dt}_{ib}")
                        nc.vector.tensor_scalar_mul(
                            osb[:], pos[ib][:], rinv[:, ib:ib + 1]
                        )
                        nc.scalar.dma_start(
                            out=o_dram[ib * 128:(ib + 1) * 128,
                                       dt * 512:(dt + 1) * 512],
                            in_=osb[:],
                        )
            for st in (eTstack, _pstack, _ostack, _vstack):
                st.__exit__(None, None, None)

    nc.compile()
    return nc


_NC_CACHE = None


def _get_nc():
    global _NC_CACHE
    if _NC_CACHE is None:
        _NC_CACHE = _build_nc()
    return _NC_CACHE


def _make_in_maps(x: np.ndarray) -> list:
    x = np.asarray(x)
    n, c, h, w = x.shape
    assert (n, c, h * w) == (N, 3, D), f"unexpected shape {x.shape}"
    xr = np.ascontiguousarray(x.reshape(n, c, h * w).transpose(1, 0, 2))
    q_full, k, v = xr[0], xr[1], xr[2]
    # kT[jb, p, db, jj] = k[jb*128+jj, db*128+p] -- per-(jb) contiguous 2MB
    kT = np.ascontiguousarray(
        k.reshape(NJ, 128, ND, 128).transpose(0, 3, 2, 1)
    )
    in_maps = []
    for core in range(N_CORES):
        qc = q_full[core * M:(core + 1) * M]          # [M, D]
        # qT[p, db, i] = q[i, db*128+p]
        qTc = np.ascontiguousarray(
            qc.reshape(M, ND, 128).transpose(2, 1, 0)
        )
        in_maps.append({"qT": qTc, "kT": kT, "v": v})
    return in_maps


def kernel(x: np.ndarray) -> np.ndarray:
    nc = _get_nc()
    res = run_bass_kernel_spmd(nc, _make_in_maps(x), core_ids=list(range(N_CORES)))
    out = np.concatenate([r["o"] for r in res.results], axis=0)
    return out.astype(np.float32)



# revision 6
# speedup vs baseline: 1.0975x; 1.0975x over previous
"""TRN2 Bass kernel for nn_Attention_86260123173325.

Single-head attention over N=4096 tokens, feature dim HW=4096:
  q, k, v = x[:,0], x[:,1], x[:,2] reshaped to [4096, 4096]
  out = softmax(0.5 * q @ k.T) @ v

Sharding: q rows split across 8 cores (512 rows each); k, v replicated.
Host-side marshaling pre-transposes q and k into PE-ready contraction-major
layouts (the PE reduces along the partition dim), and converts v to bf16
(phase-2 value quantization contributes <1e-3 output error while halving the
v HBM stream, which paces phase 2 otherwise).

Per-core algorithm (phase-1 matmuls in f32r = TF32-like; phase 2 in bf16):
  - Phase 1, per 128-row k block j: R^T[j,:] = k_j @ q^T via 32 accumulated
    f32r matmuls. Keep R^T in SBUF (fp32), and accumulate a row statistic
    W_i = sum_j exp(0.1*R_ij - 40) via exp on ACT + ones^T @ E_t matmuls.
    The -40 bias keeps W far below ~2^64 where the HW exp/f32r/ln chain was
    observed to break.  kT blocks stream as two 1MB DMAs on the two HWDGE
    rings (sync+scalar) to hold >280GB/s; block 0 is split in four so the
    first matmul can start ~5us in.
  - shift_i = 5*(ln(W_i) + 40) >= rowmax_i; any per-row shift cancels in the
    final normalization, so exp(dp - shift) is an exact softmax numerator.
  - Bridge: a few throwaway matmuls keep the PE busy through the ln->shift
    serial chain so the HAM clock gate never re-throttles (a >3.4us PE idle
    gap costs ~35us of half-clock execution afterwards).
  - Pass 2 (bf16): eT = exp(0.5*R - shift), emitted in 256-wide half tiles
    so the first phase-2 matmul trails phase 1 by ~2us.
  - Phase 2: O = (E @ v) * (1/rowsum), j-outer so each eT weight load feeds
    2 column-block matmuls, in five passes that fit the 8 PSUM banks:
      A: rowsum(ib0,ib1) + O[ib0/1, cols 0:1024]   (races the exp pass)
      B: rowsum(ib2,ib3) + O[ib2/3, cols 0:1024]   (v tiles reused from A)
      then six passes O[all ib, one 512-col block each] for cols 1024:4096
      (4 of 6 shared PSUM banks active, 2 spare so passes overlap).
    Rowsums ride the same weight loads; reciprocals are computed per ib-pair
    so pass-A PSUM banks release before pass B needs them.
"""
import sys

sys.path.insert(0, "/opt/trn_rl_repo")

import ml_dtypes
import numpy as np

import concourse.bass as bass
import concourse.tile as tile
from concourse import bacc, mybir
from concourse.bass_utils import run_bass_kernel_spmd

F32 = mybir.dt.float32
F32R = mybir.dt.float32r
BF16 = mybir.dt.bfloat16
EXP = mybir.ActivationFunctionType.Exp
LN = mybir.ActivationFunctionType.Ln

N_CORES = 8
N = 4096          # tokens (keys)
D = 4096          # feature dim (H*W)
M = N // N_CORES  # q rows per core = 512
NJ = N // 128     # 32 key blocks
ND = D // 128     # 32 feature blocks
NI = M // 128     # 4 q-row blocks per core
T_STAT = 0.2      # stage-1 temperature: exp(t*dp - 40) = exp(0.1*R - 40)
STAT_BIAS = 40.0
N_BRIDGE = 10     # PE keep-warm matmuls across the softmax serial chain


def _build_nc():
    nc = bacc.Bacc(None, target_bir_lowering=False, debug=False)

    # qT[p, db, i] = q[i, db*128+p]; kT[jb, p, db, jj] = k[jb*128+jj, db*128+p]
    qT_dram = nc.dram_tensor("qT", [128, ND, M], F32R, kind="ExternalInput")
    kT_dram = nc.dram_tensor("kT", [NJ, 128, ND, 128], F32R, kind="ExternalInput")
    v_dram = nc.dram_tensor("v", [N, D], BF16, kind="ExternalInput")
    o_dram = nc.dram_tensor("o", [M, D], F32, kind="ExternalOutput")

    with tile.TileContext(nc) as tc:
        with tc.tile_pool(name="persist", bufs=1) as persist:
            # R^T storage, [j-within-block, j-block, i] (fp32, exact scores)
            s_sb = persist.tile([128, NJ, M], F32)

            ones_f = persist.tile([128, 128], F32, tag="ones_f")
            nc.vector.memset(ones_f[:], 1.0)
            # all-ones f32r [128,128]: W-stat lhsT (output lands broadcast on
            # all 128 partitions) + bridge matmuls
            ones_r = persist.tile([128, 128], F32R, tag="ones_r")
            nc.vector.tensor_copy(ones_r[:], ones_f[:])
            # bf16 ones [128,2]: phase-2 rowsum rhs
            ones_h = persist.tile([128, 2], BF16, tag="ones_h")
            nc.vector.tensor_copy(ones_h[:], ones_f[:, 0:2])

            zero_b = persist.tile([128, 1], F32, tag="zero_b")
            nc.vector.memset(zero_b[:], 0.0)

            # stage-1 exp bias: keeps W = sum exp(0.2*dp - 40) well under
            # ~2^64, where the HW exp/f32r-matmul/ln chain breaks
            stat_b = persist.tile([128, 1], F32, tag="stat_b")
            nc.vector.memset(stat_b[:], -STAT_BIAS)

            sh2_bc = persist.tile([128, M], F32, tag="sh2_bc")
            w_ln = persist.tile([128, M], F32, tag="w_ln")
            rsum = persist.tile([128, NI], F32, tag="rsum")
            rinv = persist.tile([128, NI], F32, tag="rinv")
            br_sb = persist.tile([128, 2], F32, tag="br_sb")

            # ---------------- phase 1: R^T blocks + W stats ----------------
            with (
                tc.tile_pool(name="qT", bufs=1) as qTpool,
                tc.tile_pool(name="kT", bufs=3) as kTpool,
                tc.tile_pool(name="ett", bufs=2) as etpool,
                tc.tile_pool(name="psS", bufs=2, space="PSUM") as psS,
                tc.tile_pool(name="psW", bufs=1, space="PSUM") as psWp,
                tc.tile_pool(name="psBr", bufs=1, space="PSUM") as psBr,
            ):
                # 8 chunk tiles -> fine-grained deps; matmuls start after
                # the first 1MB chunk instead of the full 8.4MB load
                qT_parts = []
                for b in range(8):
                    qp = qTpool.tile([128, ND // 8, M], F32R, tag=f"qT{b}",
                                     name=f"qT{b}")
                    nc.scalar.dma_start(
                        out=qp[:], in_=qT_dram[:, b * (ND // 8):(b + 1) * (ND // 8), :]
                    )
                    qT_parts.append(qp)

                def qT_slice(dblk):
                    return qT_parts[dblk // (ND // 8)][:, dblk % (ND // 8), :]

                psW = psWp.tile([128, M], F32)
                ett31 = None
                for j in range(NJ):
                    kT = kTpool.tile([128, ND, 128], F32R, tag="kT",
                                     name=f"kT{j}")
                    # split across both HWDGE rings; block 0 in quarters so
                    # the first matmul starts as early as possible
                    nsl = 4 if j == 0 else 2
                    step = ND // nsl
                    for s in range(nsl):
                        eng = nc.sync if s % 2 == 0 else nc.scalar
                        eng.dma_start(
                            out=kT[:, s * step:(s + 1) * step, :],
                            in_=kT_dram[j][:, s * step:(s + 1) * step, :],
                        )
                    ps = psS.tile([128, M], F32, tag="S", name=f"ps{j}")
                    for dblk in range(ND):
                        nc.tensor.matmul(
                            ps[:],
                            kT[:, dblk, :],
                            qT_slice(dblk),
                            start=(dblk == 0),
                            stop=(dblk == ND - 1),
                        )
                    # stash raw scores R^T (fp32)
                    nc.vector.tensor_copy(s_sb[:, j, :], ps[:])
                    # W stat: exp(0.1*R - 40) then ones^T @ E_t -> psW
                    ett = etpool.tile([128, M], F32R, tag="ett", name=f"et{j}")
                    nc.scalar.activation(
                        out=ett[:], in_=ps[:], func=EXP,
                        bias=stat_b[:], scale=0.5 * T_STAT,
                    )
                    nc.tensor.matmul(
                        psW[:],
                        ones_r[:],
                        ett[:],
                        start=(j == 0),
                        stop=(j == NJ - 1),
                        skip_group_check=True,
                    )
                    if j == NJ - 1:
                        ett31 = ett

                # PE bridge across the ln->shift chain: gated on ett31 so the
                # scheduler cannot run these early, overwritten in place so
                # they serialize back-to-back on the PE
                br = psBr.tile([128, M], F32)
                for r in range(N_BRIDGE):
                    nc.tensor.matmul(
                        br[:], ones_r[:], ett31[:],
                        start=True, stop=True, skip_group_check=True,
                    )
                nc.vector.tensor_copy(br_sb[:], br[:, 0:2])

                # sh2 = 2*shift = (2/t)*(lnW' + 40); psW rows are identical
                # (all-ones lhsT) so this lands already broadcast.  Half
                # tiles shorten the serial chain ahead of the first phase-2
                # consumer.
                for h in range(2):
                    hs = slice(h * (M // 2), (h + 1) * (M // 2))
                    nc.scalar.activation(
                        out=w_ln[:, hs], in_=psW[:, hs], func=LN,
                        bias=zero_b[:], scale=1.0,
                    )
                    nc.vector.tensor_scalar(
                        sh2_bc[:, hs], w_ln[:, hs], 2.0 / T_STAT,
                        STAT_BIAS * 2.0 / T_STAT,
                        mybir.AluOpType.mult, mybir.AluOpType.add,
                    )

            # ---------------- phase 2: eT = exp(0.5*R - shift); O ----------
            with (
                tc.tile_pool(name="eTp", bufs=1) as eTpool,
                tc.tile_pool(name="tmp", bufs=4) as tmpool,
                tc.tile_pool(name="vq0", bufs=1) as vq0pool,
                tc.tile_pool(name="vrot", bufs=6) as vpool,
                tc.tile_pool(name="osb", bufs=6) as opool,
                tc.tile_pool(name="psO", bufs=6, space="PSUM") as psO,
            ):
                eT_t = eTpool.tile([128, NJ, M], BF16, name="eT_t")

                def eT(j, i0, i1):
                    return eT_t[:, j, i0:i1]

                def scale_store(ps_tile, ib, c0, label):
                    osb = opool.tile([128, 512], F32, tag="osb",
                                     name=f"ob_{label}")
                    nc.vector.tensor_scalar_mul(
                        osb[:], ps_tile[:], rinv[:, ib:ib + 1]
                    )
                    nc.scalar.dma_start(
                        out=o_dram[ib * 128:(ib + 1) * 128, c0:c0 + 512],
                        in_=osb[:],
                    )

                # ---- passes A/B need 2 extra banks for the rowsum tiles
                psR_stack = tc.tile_pool(name="psR", bufs=2, space="PSUM")
                psR = psR_stack.__enter__()

                # ---- pass A: exp pass + rowsum(ib0/1) + O[ib0/1, 0:1024]
                prA = [psR.tile([128, 2], F32, tag="pr", name=f"prA{ib}")
                       for ib in range(2)]
                oa = [psO.tile([128, 512], F32, tag="o", name=f"oa{t}")
                      for t in range(4)]
                vq_tiles = []
                for j in range(NJ):
                    vj = vq0pool.tile([128, 1024], BF16, tag=f"vq{j}",
                                      name=f"vq{j}")
                    nc.sync.dma_start(
                        out=vj[:], in_=v_dram[j * 128:(j + 1) * 128, 0:1024]
                    )
                    vq_tiles.append(vj)
                    for h in range(2):
                        hs = slice(h * (M // 2), (h + 1) * (M // 2))
                        tmp = tmpool.tile([128, M // 2], F32, tag="tmp",
                                          name=f"tmp{j}_{h}")
                        nc.vector.tensor_sub(tmp[:], s_sb[:, j, hs],
                                             sh2_bc[:, hs])
                        nc.scalar.activation(
                            out=eT_t[:, j, hs], in_=tmp[:],
                            func=EXP, bias=zero_b[:], scale=0.5,
                        )
                    for ib in range(2):
                        eTb = eT(j, ib * 128, (ib + 1) * 128)
                        nc.tensor.matmul(
                            prA[ib][:], eTb, ones_h[:],
                            start=(j == 0), stop=(j == NJ - 1),
                        )
                        for dq in range(2):
                            nc.tensor.matmul(
                                oa[ib * 2 + dq][:], eTb,
                                vj[:, dq * 512:(dq + 1) * 512],
                                start=(j == 0), stop=(j == NJ - 1),
                            )
                for ib in range(2):
                    nc.vector.tensor_copy(rsum[:, ib:ib + 1], prA[ib][:, 0:1])
                nc.vector.reciprocal(rinv[:, 0:2], rsum[:, 0:2])
                for ib in range(2):
                    for dq in range(2):
                        scale_store(oa[ib * 2 + dq], ib, dq * 512,
                                    f"A{ib}_{dq}")

                # ---- pass B: rowsum(ib2/3) + O[ib2/3, 0:1024], v reused
                prB = [psR.tile([128, 2], F32, tag="pr", name=f"prB{ib}")
                       for ib in range(2)]
                ob = [psO.tile([128, 512], F32, tag="o", name=f"obt{t}")
                      for t in range(4)]
                for j in range(NJ):
                    vj = vq_tiles[j]
                    for ib in range(2, 4):
                        eTb = eT(j, ib * 128, (ib + 1) * 128)
                        nc.tensor.matmul(
                            prB[ib - 2][:], eTb, ones_h[:],
                            start=(j == 0), stop=(j == NJ - 1),
                        )
                        for dq in range(2):
                            nc.tensor.matmul(
                                ob[(ib - 2) * 2 + dq][:], eTb,
                                vj[:, dq * 512:(dq + 1) * 512],
                                start=(j == 0), stop=(j == NJ - 1),
                            )
                for ib in range(2, 4):
                    nc.vector.tensor_copy(rsum[:, ib:ib + 1],
                                          prB[ib - 2][:, 0:1])
                nc.vector.reciprocal(rinv[:, 2:4], rsum[:, 2:4])
                for ib in range(2, 4):
                    for dq in range(2):
                        scale_store(ob[(ib - 2) * 2 + dq], ib, dq * 512,
                                    f"B{ib}_{dq}")

                psR_stack.__exit__(None, None, None)

                # ---- six passes: O[all ib, one 512-col block], 1024:4096
                for p in range(6):
                    c0 = 1024 + p * 512
                    oc = [psO.tile([128, 512], F32, tag="o",
                                   name=f"oc{p}_{t}") for t in range(NI)]
                    for j in range(NJ):
                        vj = vpool.tile([128, 512], BF16, tag="v",
                                        name=f"v{p}_{j}")
                        nc.sync.dma_start(
                            out=vj[:],
                            in_=v_dram[j * 128:(j + 1) * 128, c0:c0 + 512],
                        )
                        for ib in range(NI):
                            nc.tensor.matmul(
                                oc[ib][:], eT(j, ib * 128, (ib + 1) * 128),
                                vj[:],
                                start=(j == 0), stop=(j == NJ - 1),
                            )
                    for ib in range(NI):
                        scale_store(oc[ib], ib, c0, f"C{p}_{ib}")

    nc.compile()
    return nc


_NC_CACHE = None


def _get_nc():
    global _NC_CACHE
    if _NC_CACHE is None:
        _NC_CACHE = _build_nc()
    return _NC_CACHE


def _make_in_maps(x: np.ndarray) -> list:
    x = np.asarray(x)
    n, c, h, w = x.shape
    assert (n, c, h * w) == (N, 3, D), f"unexpected shape {x.shape}"
    xr = np.ascontiguousarray(x.reshape(n, c, h * w).transpose(1, 0, 2))
    q_full, k, v = xr[0], xr[1], xr[2]
    # kT[jb, p, db, jj] = k[jb*128+jj, db*128+p] -- per-(jb) contiguous 2MB
    kT = np.ascontiguousarray(
        k.reshape(NJ, 128, ND, 128).transpose(0, 3, 2, 1)
    )
    v_bf16 = v.astype(ml_dtypes.bfloat16)
    in_maps = []
    for core in range(N_CORES):
        qc = q_full[core * M:(core + 1) * M]          # [M, D]
        # qT[p, db, i] = q[i, db*128+p]
        qTc = np.ascontiguousarray(
            qc.reshape(M, ND, 128).transpose(2, 1, 0)
        )
        in_maps.append({"qT": qTc, "kT": kT, "v": v_bf16})
    return in_maps


def kernel(x: np.ndarray) -> np.ndarray:
    nc = _get_nc()
    res = run_bass_kernel_spmd(nc, _make_in_maps(x), core_ids=list(range(N_CORES)))
    out = np.concatenate([r["o"] for r in res.results], axis=0)
    return out.astype(np.float32)


# revision 12
# speedup vs baseline: 1.1175x; 1.0182x over previous
"""TRN2 Bass kernel for nn_Attention_86260123173325.

Single-head attention over N=4096 tokens, feature dim HW=4096:
  q, k, v = x[:,0], x[:,1], x[:,2] reshaped to [4096, 4096]
  out = softmax(0.5 * q @ k.T) @ v

Sharding: q rows split across 8 cores (512 rows each); k, v replicated.
Host-side marshaling pre-transposes q and k into PE-ready contraction-major
layouts (the PE reduces along the partition dim), and converts v to bf16
(phase-2 value quantization contributes <1e-3 output error while halving the
v HBM stream, which paces phase 2 otherwise).

Per-core algorithm (phase-1 matmuls in f32r = TF32-like; phase 2 in bf16):
  - Phase 1, per 128-row k block j: R^T[j,:] = k_j @ q^T via 32 accumulated
    f32r matmuls. Keep R^T in SBUF (fp32), and accumulate a row statistic
    W_i = sum_j exp(0.1*R_ij - 40) via exp on ACT + ones^T @ E_t matmuls.
    The -40 bias keeps W far below ~2^64 where the HW exp/f32r/ln chain was
    observed to break.  kT blocks stream as two 1MB DMAs on the two HWDGE
    rings (sync+scalar) to hold >280GB/s; block 0 is split in four so the
    first matmul can start ~5us in.
  - shift_i = 5*(ln(W_i) + 40) >= rowmax_i; any per-row shift cancels in the
    final normalization, so exp(dp - shift) is an exact softmax numerator.
  - Bridge: a few throwaway matmuls keep the PE busy through the ln->shift
    serial chain so the HAM clock gate never re-throttles (a >3.4us PE idle
    gap costs ~35us of half-clock execution afterwards).
  - Pass 2 (bf16): eT = exp(0.5*R - shift), emitted in 256-wide half tiles
    so the first phase-2 matmul trails phase 1 by ~2us.
  - Phase 2: O = (E @ v) * (1/rowsum), j-outer so each eT weight load feeds
    2 column-block matmuls, in five passes that fit the 8 PSUM banks:
      A: rowsum(ib0,ib1) + O[ib0/1, cols 0:1024]   (races the exp pass)
      B: rowsum(ib2,ib3) + O[ib2/3, cols 0:1024]   (v tiles reused from A)
      then six passes O[all ib, one 512-col block each] for cols 1024:4096
      (4 of 6 shared PSUM banks active, 2 spare so passes overlap).
    Rowsums ride the same weight loads; reciprocals are computed per ib-pair
    so pass-A PSUM banks release before pass B needs them.
"""
import sys

sys.path.insert(0, "/opt/trn_rl_repo")

import ml_dtypes
import numpy as np

import concourse.bass as bass
import concourse.tile as tile
from concourse import bacc, mybir
from concourse.bass_utils import run_bass_kernel_spmd

F32 = mybir.dt.float32
F32R = mybir.dt.float32r
BF16 = mybir.dt.bfloat16
EXP = mybir.ActivationFunctionType.Exp
LN = mybir.ActivationFunctionType.Ln

N_CORES = 8
N = 4096          # tokens (keys)
D = 4096          # feature dim (H*W)
M = N // N_CORES  # q rows per core = 512
NJ = N // 128     # 32 key blocks
ND = D // 128     # 32 feature blocks
NI = M // 128     # 4 q-row blocks per core
T_STAT = 0.2      # stage-1 temperature: exp(t*dp - 40) = exp(0.1*R - 40)
STAT_BIAS = 40.0
N_BRIDGE = 10     # PE keep-warm matmuls across the softmax serial chain


def _build_nc():
    nc = bacc.Bacc(None, target_bir_lowering=False, debug=False)

    # qT[p, db, i] = q[i, db*128+p]; kT[jb, p, db, jj] = k[jb*128+jj, db*128+p]
    qT_dram = nc.dram_tensor("qT", [128, ND, M], F32R, kind="ExternalInput")
    kT_dram = nc.dram_tensor("kT", [NJ, 128, ND, 128], F32R, kind="ExternalInput")
    v_dram = nc.dram_tensor("v", [N, D], BF16, kind="ExternalInput")
    o_dram = nc.dram_tensor("o", [M, D], F32, kind="ExternalOutput")

    with tile.TileContext(nc) as tc:
        with tc.tile_pool(name="persist", bufs=1) as persist:
            # R^T storage, [j-within-block, j-block, i] (fp32, exact scores)
            s_sb = persist.tile([128, NJ, M], F32)

            ones_f = persist.tile([128, 128], F32, tag="ones_f")
            nc.vector.memset(ones_f[:], 1.0)
            # all-ones f32r [128,128]: W-stat lhsT (output lands broadcast on
            # all 128 partitions) + bridge matmuls
            ones_r = persist.tile([128, 128], F32R, tag="ones_r")
            nc.vector.tensor_copy(ones_r[:], ones_f[:])
            # bf16 ones [128,2]: phase-2 rowsum rhs
            ones_h = persist.tile([128, 2], BF16, tag="ones_h")
            nc.vector.tensor_copy(ones_h[:], ones_f[:, 0:2])

            zero_b = persist.tile([128, 1], F32, tag="zero_b")
            nc.vector.memset(zero_b[:], 0.0)

            # stage-1 exp bias: keeps W = sum exp(0.2*dp - 40) well under
            # ~2^64, where the HW exp/f32r-matmul/ln chain breaks
            stat_b = persist.tile([128, 1], F32, tag="stat_b")
            nc.vector.memset(stat_b[:], -STAT_BIAS)

            sh2_bc = persist.tile([128, M], F32, tag="sh2_bc")
            w_ln = persist.tile([128, M], F32, tag="w_ln")
            rsum = persist.tile([128, NI], F32, tag="rsum")
            rinv = persist.tile([128, NI], F32, tag="rinv")
            br_sb = persist.tile([128, 2], F32, tag="br_sb")

            # ---------------- phase 1: R^T blocks + W stats ----------------
            with (
                tc.tile_pool(name="qT", bufs=1) as qTpool,
                tc.tile_pool(name="kT", bufs=3) as kTpool,
                tc.tile_pool(name="ett", bufs=2) as etpool,
                tc.tile_pool(name="psS", bufs=2, space="PSUM") as psS,
                tc.tile_pool(name="psW", bufs=1, space="PSUM") as psWp,
                tc.tile_pool(name="psBr", bufs=1, space="PSUM") as psBr,
            ):
                # qT in 8 chunk tiles -> fine-grained deps.  DMA emission is
                # interleaved with the kT stream in need-order across both
                # HWDGE rings: block-0 kT entirely on sync (so the first
                # matmul starts ~4us in), qT chunks alternating rings so
                # chunk b arrives about when block 0's matmul dblk=4b runs.
                qT_parts = [
                    qTpool.tile([128, ND // 8, M], F32R, tag=f"qT{b}",
                                name=f"qT{b}")
                    for b in range(8)
                ]

                def qT_dma(b, eng):
                    eng.dma_start(
                        out=qT_parts[b][:],
                        in_=qT_dram[:, b * (ND // 8):(b + 1) * (ND // 8), :],
                    )

                def qT_slice(dblk):
                    return qT_parts[dblk // (ND // 8)][:, dblk % (ND // 8), :]

                kT_tiles = {}

                def kT_dma(j, half, eng, quarters=False):
                    if j not in kT_tiles:
                        kT_tiles[j] = kTpool.tile([128, ND, 128], F32R,
                                                  tag="kT", name=f"kT{j}")
                    kt = kT_tiles[j]
                    nsl = 4 if quarters else 2
                    step = ND // nsl
                    rng = range(0, nsl, 1) if half is None else [half]
                    for s in rng:
                        eng.dma_start(
                            out=kt[:, s * step:(s + 1) * step, :],
                            in_=kT_dram[j][:, s * step:(s + 1) * step, :],
                        )

                qT_dma(0, nc.scalar)
                kT_dma(0, None, nc.sync, quarters=True)
                for b in range(1, 8):
                    qT_dma(b, nc.scalar if b % 2 == 0 else nc.sync)

                psW = psWp.tile([128, M], F32)
                # wacc[jj, i] = sum_{j-blocks} exp(0.1*R - 40); the
                # cross-partition reduction to W happens in one matmul
                wacc = persist.tile([128, M], F32, tag="wacc")
                ett31 = None
                for j in range(NJ):
                    if j > 0:
                        kT_dma(j, 0, nc.sync)
                        kT_dma(j, 1, nc.scalar)
                    kT = kT_tiles[j]
                    ps = psS.tile([128, M], F32, tag="S", name=f"ps{j}")
                    for dblk in range(ND):
                        nc.tensor.matmul(
                            ps[:],
                            kT[:, dblk, :],
                            qT_slice(dblk),
                            start=(dblk == 0),
                            stop=(dblk == ND - 1),
                        )
                    # stash raw scores R^T (fp32)
                    nc.vector.tensor_copy(s_sb[:, j, :], ps[:])
                    # W stat: exp(0.1*R - 40), accumulated on the DVE
                    ett = etpool.tile([128, M], F32R, tag="ett", name=f"et{j}")
                    nc.scalar.activation(
                        out=ett[:], in_=ps[:], func=EXP,
                        bias=stat_b[:], scale=0.5 * T_STAT,
                    )
                    if j == 0:
                        nc.gpsimd.tensor_copy(wacc[:], ett[:])
                    else:
                        nc.gpsimd.tensor_add(wacc[:], wacc[:], ett[:])
                    if j == NJ - 1:
                        ett31 = ett

                # PE bridge across the wacc/ln/shift serial chain: gated on
                # ett31 so the scheduler cannot run these early, overwritten
                # in place so they serialize back-to-back on the PE
                br = psBr.tile([128, M], F32)
                for r in range(N_BRIDGE):
                    nc.tensor.matmul(
                        br[:], ones_r[:], ett31[:],
                        start=True, stop=True, skip_group_check=True,
                    )
                nc.vector.tensor_copy(br_sb[:], br[:, 0:2])

                # cross-partition reduce: psW[p, i] = W_i (broadcast), f32
                nc.tensor.matmul(psW[:], ones_f[:], wacc[:],
                                 start=True, stop=True)

                # sh2 = 2*shift = (2/t)*(lnW' + 40); psW rows are identical
                # (all-ones lhsT) so this lands already broadcast.  Half
                # tiles shorten the serial chain ahead of the first phase-2
                # consumer.
                for h in range(2):
                    hs = slice(h * (M // 2), (h + 1) * (M // 2))
                    nc.scalar.activation(
                        out=w_ln[:, hs], in_=psW[:, hs], func=LN,
                        bias=zero_b[:], scale=1.0,
                    )
                    nc.vector.tensor_scalar(
                        sh2_bc[:, hs], w_ln[:, hs], 2.0 / T_STAT,
                        STAT_BIAS * 2.0 / T_STAT,
                        mybir.AluOpType.mult, mybir.AluOpType.add,
                    )

            # ---------------- phase 2: eT = exp(0.5*R - shift); O ----------
            with (
                tc.tile_pool(name="eTp", bufs=1) as eTpool,
                tc.tile_pool(name="tmp", bufs=4) as tmpool,
                tc.tile_pool(name="vq0", bufs=1) as vq0pool,
                tc.tile_pool(name="vrot", bufs=6) as vpool,
                tc.tile_pool(name="osb", bufs=6) as opool,
                tc.tile_pool(name="psO", bufs=7, space="PSUM") as psO,
                tc.tile_pool(name="psR", bufs=1, space="PSUM") as psR,
            ):
                eT_t = eTpool.tile([128, NJ, M], BF16, name="eT_t")

                def eT(j, i0, i1):
                    return eT_t[:, j, i0:i1]

                def scale_store(ps_tile, ib, c0, label):
                    osb = opool.tile([128, 512], F32, tag="osb",
                                     name=f"ob_{label}")
                    nc.vector.tensor_scalar_mul(
                        osb[:], ps_tile[:], rinv[:, ib:ib + 1]
                    )
                    nc.scalar.dma_start(
                        out=o_dram[ib * 128:(ib + 1) * 128, c0:c0 + 512],
                        in_=osb[:],
                    )

                # rowsum accumulator: acc[jj, i] = sum_{j-blocks} eT (f32 so
                # rounding doesn't bias the normalization)
                acc = persist.tile([128, M], F32, tag="acc")

                # ---- pass A: exp pass + rowsum accum + O[ib0/1, 0:1024]
                oa = [psO.tile([128, 512], F32, tag="o", name=f"oa{t}")
                      for t in range(4)]
                vq_tiles = []
                for j in range(NJ):
                    vj = vq0pool.tile([128, 1024], BF16, tag=f"vq{j}",
                                      name=f"vq{j}")
                    nc.sync.dma_start(
                        out=vj[:], in_=v_dram[j * 128:(j + 1) * 128, 0:1024]
                    )
                    vq_tiles.append(vj)
                    for h in range(2):
                        hs = slice(h * (M // 2), (h + 1) * (M // 2))
                        tmp = tmpool.tile([128, M // 2], F32, tag="tmp",
                                          name=f"tmp{j}_{h}")
                        nc.vector.tensor_sub(tmp[:], s_sb[:, j, hs],
                                             sh2_bc[:, hs])
                        nc.scalar.activation(
                            out=eT_t[:, j, hs], in_=tmp[:],
                            func=EXP, bias=zero_b[:], scale=0.5,
                        )
                    if j == 0:
                        nc.gpsimd.tensor_copy(acc[:], eT_t[:, 0, :])
                    else:
                        nc.gpsimd.tensor_add(acc[:], acc[:], eT_t[:, j, :])
                    for ib in range(2):
                        eTb = eT(j, ib * 128, (ib + 1) * 128)
                        for dq in range(2):
                            nc.tensor.matmul(
                                oa[ib * 2 + dq][:], eTb,
                                vj[:, dq * 512:(dq + 1) * 512],
                                start=(j == 0), stop=(j == NJ - 1),
                            )
                # rowsums: 4 tiny f32 matmuls into column slices of one
                # PSUM bank (disjoint has_written ranges)
                pr = psR.tile([128, 8], F32, name="pr")
                for ib in range(NI):
                    nc.tensor.matmul(
                        pr[:, 2 * ib:2 * ib + 2],
                        acc[:, ib * 128:(ib + 1) * 128],
                        ones_f[:, 0:2],
                        start=True, stop=True, skip_group_check=True,
                    )
                    nc.vector.tensor_copy(rsum[:, ib:ib + 1],
                                          pr[:, 2 * ib:2 * ib + 1])
                nc.vector.reciprocal(rinv[:], rsum[:])
                for ib in range(2):
                    for dq in range(2):
                        scale_store(oa[ib * 2 + dq], ib, dq * 512,
                                    f"A{ib}_{dq}")

                # ---- pass B: O[ib2/3, 0:1024], v tiles reused from A
                ob = [psO.tile([128, 512], F32, tag="o", name=f"obt{t}")
                      for t in range(4)]
                for j in range(NJ):
                    vj = vq_tiles[j]
                    for ib in range(2, 4):
                        eTb = eT(j, ib * 128, (ib + 1) * 128)
                        for dq in range(2):
                            nc.tensor.matmul(
                                ob[(ib - 2) * 2 + dq][:], eTb,
                                vj[:, dq * 512:(dq + 1) * 512],
                                start=(j == 0), stop=(j == NJ - 1),
                            )
                for ib in range(2, 4):
                    for dq in range(2):
                        scale_store(ob[(ib - 2) * 2 + dq], ib, dq * 512,
                                    f"B{ib}_{dq}")

                # ---- six passes: O[all ib, one 512-col block], 1024:4096
                for p in range(6):
                    c0 = 1024 + p * 512
                    oc = [psO.tile([128, 512], F32, tag="o",
                                   name=f"oc{p}_{t}") for t in range(NI)]
                    for j in range(NJ):
                        vj = vpool.tile([128, 512], BF16, tag="v",
                                        name=f"v{p}_{j}")
                        nc.sync.dma_start(
                            out=vj[:],
                            in_=v_dram[j * 128:(j + 1) * 128, c0:c0 + 512],
                        )
                        for ib in range(NI):
                            nc.tensor.matmul(
                                oc[ib][:], eT(j, ib * 128, (ib + 1) * 128),
                                vj[:],
                                start=(j == 0), stop=(j == NJ - 1),
                            )
                    for ib in range(NI):
                        scale_store(oc[ib], ib, c0, f"C{p}_{ib}")

    nc.compile()
    return nc


_NC_CACHE = None


def _get_nc():
    global _NC_CACHE
    if _NC_CACHE is None:
        _NC_CACHE = _build_nc()
    return _NC_CACHE


def _make_in_maps(x: np.ndarray) -> list:
    x = np.asarray(x)
    n, c, h, w = x.shape
    assert (n, c, h * w) == (N, 3, D), f"unexpected shape {x.shape}"
    xr = np.ascontiguousarray(x.reshape(n, c, h * w).transpose(1, 0, 2))
    q_full, k, v = xr[0], xr[1], xr[2]
    # kT[jb, p, db, jj] = k[jb*128+jj, db*128+p] -- per-(jb) contiguous 2MB
    kT = np.ascontiguousarray(
        k.reshape(NJ, 128, ND, 128).transpose(0, 3, 2, 1)
    )
    v_bf16 = v.astype(ml_dtypes.bfloat16)
    in_maps = []
    for core in range(N_CORES):
        qc = q_full[core * M:(core + 1) * M]          # [M, D]
        # qT[p, db, i] = q[i, db*128+p]
        qTc = np.ascontiguousarray(
            qc.reshape(M, ND, 128).transpose(2, 1, 0)
        )
        in_maps.append({"qT": qTc, "kT": kT, "v": v_bf16})
    return in_maps


def kernel(x: np.ndarray) -> np.ndarray:
    nc = _get_nc()
    res = run_bass_kernel_spmd(nc, _make_in_maps(x), core_ids=list(range(N_CORES)))
    out = np.concatenate([r["o"] for r in res.results], axis=0)
    return out.astype(np.float32)


# revision 19
# speedup vs baseline: 1.1524x; 1.0312x over previous
"""TRN2 Bass kernel for nn_Attention_86260123173325.

Single-head attention over N=4096 tokens, feature dim HW=4096:
  q, k, v = x[:,0], x[:,1], x[:,2] reshaped to [4096, 4096]
  out = softmax(0.5 * q @ k.T) @ v

Sharding: q rows split across 8 cores (512 rows each); k, v replicated.
Host-side marshaling pre-transposes q and k into PE-ready contraction-major
layouts (the PE reduces along the partition dim), and converts v to bf16
(phase-2 value quantization contributes <1e-3 output error while halving the
v HBM stream, which paces phase 2 otherwise).

Per-core algorithm (phase-1 matmuls in f32r = TF32-like; phase 2 in bf16):
  - Phase 1, per 128-row k block j: R^T[j,:] = k_j @ q^T via 32 accumulated
    f32r matmuls. Keep R^T in SBUF (fp32), and accumulate a row statistic
    W_i = sum_j exp(0.1*R_ij - 40) via exp on ACT + ones^T @ E_t matmuls.
    The -40 bias keeps W far below ~2^64 where the HW exp/f32r/ln chain was
    observed to break.  kT blocks stream as two 1MB DMAs on the two HWDGE
    rings (sync+scalar) to hold >280GB/s; block 0 is split in four so the
    first matmul can start ~5us in.
  - shift_i = 5*(ln(W_i) + 40) >= rowmax_i; any per-row shift cancels in the
    final normalization, so exp(dp - shift) is an exact softmax numerator.
  - Bridge: a few throwaway matmuls keep the PE busy through the ln->shift
    serial chain so the HAM clock gate never re-throttles (a >3.4us PE idle
    gap costs ~35us of half-clock execution afterwards).
  - Pass 2 (bf16): eT = exp(0.5*R - shift), emitted in 256-wide half tiles
    so the first phase-2 matmul trails phase 1 by ~2us.
  - Phase 2: O = (E @ v) * (1/rowsum), j-outer so each eT weight load feeds
    2 column-block matmuls, in five passes that fit the 8 PSUM banks:
      A: rowsum(ib0,ib1) + O[ib0/1, cols 0:1024]   (races the exp pass)
      B: rowsum(ib2,ib3) + O[ib2/3, cols 0:1024]   (v tiles reused from A)
      then six passes O[all ib, one 512-col block each] for cols 1024:4096
      (4 of 6 shared PSUM banks active, 2 spare so passes overlap).
    Rowsums ride the same weight loads; reciprocals are computed per ib-pair
    so pass-A PSUM banks release before pass B needs them.
"""
import sys

sys.path.insert(0, "/opt/trn_rl_repo")

import ml_dtypes
import numpy as np

import concourse.bass as bass
import concourse.tile as tile
from concourse import bacc, mybir
from concourse.bass_utils import run_bass_kernel_spmd

F32 = mybir.dt.float32
F32R = mybir.dt.float32r
BF16 = mybir.dt.bfloat16
EXP = mybir.ActivationFunctionType.Exp
LN = mybir.ActivationFunctionType.Ln

N_CORES = 8
N = 4096          # tokens (keys)
D = 4096          # feature dim (H*W)
M = N // N_CORES  # q rows per core = 512
NJ = N // 128     # 32 key blocks
ND = D // 128     # 32 feature blocks
NI = M // 128     # 4 q-row blocks per core
T_STAT = 0.2      # stage-1 temperature: exp(t*dp - 40) = exp(0.1*R - 40)
STAT_BIAS = 40.0
N_BRIDGE = 10     # PE keep-warm matmuls across the softmax serial chain


def _build_nc():
    nc = bacc.Bacc(None, target_bir_lowering=False, debug=False)

    # qT[p, db, i] = q[i, db*128+p]; kT[jb, p, db, jj] = k[jb*128+jj, db*128+p]
    qT_dram = nc.dram_tensor("qT", [128, ND, M], F32R, kind="ExternalInput")
    kT_dram = nc.dram_tensor("kT", [NJ, 128, ND, 128], F32R, kind="ExternalInput")
    v_dram = nc.dram_tensor("v", [N, D], BF16, kind="ExternalInput")
    o_dram = nc.dram_tensor("o", [M, D], F32, kind="ExternalOutput")

    with tile.TileContext(nc) as tc:
        with tc.tile_pool(name="persist", bufs=1) as persist:
            # R^T storage, [j-within-block, j-block, i] (fp32, exact scores)
            s_sb = persist.tile([128, NJ, M], F32)

            ones_f = persist.tile([128, 128], F32, tag="ones_f")
            nc.vector.memset(ones_f[:], 1.0)
            # all-ones f32r [128,128]: W-stat lhsT (output lands broadcast on
            # all 128 partitions) + bridge matmuls
            ones_r = persist.tile([128, 128], F32R, tag="ones_r")
            nc.vector.tensor_copy(ones_r[:], ones_f[:])
            # bf16 ones [128,2]: phase-2 rowsum rhs
            ones_h = persist.tile([128, 2], BF16, tag="ones_h")
            nc.vector.tensor_copy(ones_h[:], ones_f[:, 0:2])

            zero_b = persist.tile([128, 1], F32, tag="zero_b")
            nc.vector.memset(zero_b[:], 0.0)

            # stage-1 exp bias: keeps W = sum exp(0.2*dp - 40) well under
            # ~2^64, where the HW exp/f32r-matmul/ln chain breaks
            stat_b = persist.tile([128, 1], F32, tag="stat_b")
            nc.vector.memset(stat_b[:], -STAT_BIAS)

            # exp bias for pass 2: exp(0.5*(R - 10*lnW - 400))
            #                    = exp(0.5*R - 5*lnW - 200)
            e2_b = persist.tile([128, 1], F32, tag="e2_b")
            nc.vector.memset(e2_b[:], -0.5 * STAT_BIAS * 2.0 / T_STAT)
            w_ln = persist.tile([128, M], F32, tag="w_ln")
            rsum = persist.tile([128, NI], F32, tag="rsum")
            rinv = persist.tile([128, NI], F32, tag="rinv")
            br_sb = persist.tile([128, 2], F32, tag="br_sb")

            # ---------------- phase 1: R^T blocks + W stats ----------------
            with (
                tc.tile_pool(name="qT", bufs=1) as qTpool,
                tc.tile_pool(name="kT", bufs=3) as kTpool,
                tc.tile_pool(name="ett", bufs=2) as etpool,
                tc.tile_pool(name="psS", bufs=2, space="PSUM") as psS,
                tc.tile_pool(name="psW", bufs=1, space="PSUM") as psWp,
                tc.tile_pool(name="psBr", bufs=1, space="PSUM") as psBr,
            ):
                # qT in 8 chunk tiles -> fine-grained deps.  DMA emission is
                # interleaved with the kT stream in need-order across both
                # HWDGE rings: block-0 kT entirely on sync (so the first
                # matmul starts ~4us in), qT chunks alternating rings so
                # chunk b arrives about when block 0's matmul dblk=4b runs.
                qT_parts = [
                    qTpool.tile([128, ND // 8, M], F32R, tag=f"qT{b}",
                                name=f"qT{b}")
                    for b in range(8)
                ]

                def qT_dma(b, eng):
                    eng.dma_start(
                        out=qT_parts[b][:],
                        in_=qT_dram[:, b * (ND // 8):(b + 1) * (ND // 8), :],
                    )

                def qT_slice(dblk):
                    return qT_parts[dblk // (ND // 8)][:, dblk % (ND // 8), :]

                kT_tiles = {}

                def kT_dma(j, half, eng, quarters=False):
                    if j not in kT_tiles:
                        kT_tiles[j] = kTpool.tile([128, ND, 128], F32R,
                                                  tag="kT", name=f"kT{j}")
                    kt = kT_tiles[j]
                    nsl = 4 if quarters else 2
                    step = ND // nsl
                    rng = range(0, nsl, 1) if half is None else [half]
                    for s in rng:
                        eng.dma_start(
                            out=kt[:, s * step:(s + 1) * step, :],
                            in_=kT_dram[j][:, s * step:(s + 1) * step, :],
                        )

                qT_dma(0, nc.scalar)
                kT_dma(0, None, nc.sync, quarters=True)
                for b in range(1, 8):
                    qT_dma(b, nc.scalar if b % 2 == 0 else nc.sync)

                psW = psWp.tile([128, M], F32)
                # wacc[jj, i] = sum_{j-blocks} exp(0.1*R - 40); the
                # cross-partition reduction to W happens in one matmul
                wacc = persist.tile([128, M], F32, tag="wacc")
                ett31 = None
                for j in range(NJ):
                    if j > 0:
                        kT_dma(j, 0, nc.sync)
                        kT_dma(j, 1, nc.scalar)
                    kT = kT_tiles[j]
                    ps = psS.tile([128, M], F32, tag="S", name=f"ps{j}")
                    for dblk in range(ND):
                        nc.tensor.matmul(
                            ps[:],
                            kT[:, dblk, :],
                            qT_slice(dblk),
                            start=(dblk == 0),
                            stop=(dblk == ND - 1),
                        )
                    # stash raw scores R^T (fp32)
                    nc.vector.tensor_copy(s_sb[:, j, :], ps[:])
                    # W stat: exp(0.1*R - 40), accumulated on the DVE
                    ett = etpool.tile([128, M], F32R, tag="ett", name=f"et{j}")
                    nc.scalar.activation(
                        out=ett[:], in_=ps[:], func=EXP,
                        bias=stat_b[:], scale=0.5 * T_STAT,
                    )
                    if j == 0:
                        nc.gpsimd.tensor_copy(wacc[:], ett[:])
                    else:
                        nc.gpsimd.tensor_add(wacc[:], wacc[:], ett[:])
                    if j == NJ - 2:
                        ett_pin = ett

                # PE bridge across the wacc/ln/shift serial chain: gated on
                # ett30 so it starts the moment the last score matmul ends,
                # overwritten in place so it serializes back-to-back on the
                # PE.  Split around the psW reduction (which waits on the
                # gpsimd wacc chain tail) so no PE-idle window exceeds the
                # ~3.4us HAM re-throttle threshold.
                br = psBr.tile([128, M], F32)
                for r in range(N_BRIDGE):
                    nc.tensor.matmul(
                        br[:], ones_r[:], ett_pin[:],
                        start=True, stop=True, skip_group_check=True,
                    )

                # cross-partition reduce: psW[p, i] = W_i (broadcast), f32
                nc.tensor.matmul(psW[:], ones_f[:], wacc[:],
                                 start=True, stop=True)

                for r in range(6):
                    nc.tensor.matmul(
                        br[:], ones_r[:], ett_pin[:],
                        start=True, stop=True, skip_group_check=True,
                    )
                nc.vector.tensor_copy(br_sb[:], br[:, 0:2])

                # lnW' (psW rows are identical so this lands broadcast);
                # the 10*lnW+400 shift is fused into the pass-2
                # scalar_tensor_tensor + exp bias.  Half tiles shorten the
                # serial chain ahead of the first phase-2 consumer.
                for h in range(2):
                    hs = slice(h * (M // 2), (h + 1) * (M // 2))
                    nc.scalar.activation(
                        out=w_ln[:, hs], in_=psW[:, hs], func=LN,
                        bias=zero_b[:], scale=1.0,
                    )

            # ---------------- phase 2: eT = exp(0.5*R - shift); O ----------
            with (
                tc.tile_pool(name="eTp", bufs=1) as eTpool,
                tc.tile_pool(name="tmp", bufs=4) as tmpool,
                tc.tile_pool(name="vq0", bufs=1) as vq0pool,
                tc.tile_pool(name="vrot", bufs=6) as vpool,
                tc.tile_pool(name="osb", bufs=6) as opool,
                tc.tile_pool(name="psO", bufs=6, space="PSUM") as psO,
                tc.tile_pool(name="psR", bufs=2, space="PSUM") as psR,
            ):
                eT_t = eTpool.tile([128, NJ, M], BF16, name="eT_t")

                def eT(j, i0, i1):
                    return eT_t[:, j, i0:i1]

                def scale_store(ps_tile, ib, c0, label):
                    osb = opool.tile([128, 512], F32, tag="osb",
                                     name=f"ob_{label}")
                    nc.vector.tensor_scalar_mul(
                        osb[:], ps_tile[:], rinv[:, ib:ib + 1]
                    )
                    nc.scalar.dma_start(
                        out=o_dram[ib * 128:(ib + 1) * 128, c0:c0 + 512],
                        in_=osb[:],
                    )

                def exp_block(j):
                    # tmp = s_sb - 10*lnW; exp applies *0.5 and -200 bias.
                    # First two blocks in halves so the first phase-2 matmul
                    # trails the ln by ~1.5us; the rest full-width (fewer
                    # DVE/ACT dispatches).
                    nh = 2 if j < 2 else 1
                    for h in range(nh):
                        hs = slice(h * (M // nh), (h + 1) * (M // nh))
                        tmp = tmpool.tile([128, M // nh], F32, tag="tmp",
                                          name=f"tmp{j}_{h}")
                        nc.vector.scalar_tensor_tensor(
                            tmp[:], w_ln[:, hs], -2.0 / T_STAT,
                            s_sb[:, j, hs],
                            mybir.AluOpType.mult, mybir.AluOpType.add,
                        )
                        nc.scalar.activation(
                            out=eT_t[:, j, hs], in_=tmp[:],
                            func=EXP, bias=e2_b[:], scale=0.5,
                        )

                # rowsums: per-(j, ib) tiny matmuls ride the pass A/B weight
                # loads.  Separate PSUM banks per pass — sharing one bank is
                # a fatal PE-write/DVE-read bank conflict (pass B's matmuls
                # race pass A's rowsum copies).

                # ---- pass A: exp pass + rowsum(ib0/1) + O[ib0/1, 0:1024]
                prA = [psR.tile([128, 2], F32, tag="pr", name=f"prA{ib}")
                       for ib in range(2)]
                oa = [psO.tile([128, 512], F32, tag="o", name=f"oa{t}")
                      for t in range(4)]
                vq_tiles = []
                for j in range(NJ):
                    vj = vq0pool.tile([128, 1024], BF16, tag=f"vq{j}",
                                      name=f"vq{j}")
                    nc.sync.dma_start(
                        out=vj[:], in_=v_dram[j * 128:(j + 1) * 128, 0:1024]
                    )
                    vq_tiles.append(vj)
                    exp_block(j)
                    for ib in range(2):
                        eTb = eT(j, ib * 128, (ib + 1) * 128)
                        nc.tensor.matmul(
                            prA[ib][:], eTb, ones_h[:],
                            start=(j == 0), stop=(j == NJ - 1),
                        )
                        for dq in range(2):
                            nc.tensor.matmul(
                                oa[ib * 2 + dq][:], eTb,
                                vj[:, dq * 512:(dq + 1) * 512],
                                start=(j == 0), stop=(j == NJ - 1),
                            )
                for ib in range(2):
                    nc.vector.tensor_copy(rsum[:, ib:ib + 1],
                                          prA[ib][:, 0:1])
                nc.vector.reciprocal(rinv[:, 0:2], rsum[:, 0:2])
                for ib in range(2):
                    for dq in range(2):
                        scale_store(oa[ib * 2 + dq], ib, dq * 512,
                                    f"A{ib}_{dq}")

                # ---- pass B: rowsum(ib2/3) + O[ib2/3, 0:1024], v reused
                prB = [psR.tile([128, 2], F32, tag="pr", name=f"prB{ib}")
                       for ib in range(2)]
                ob = [psO.tile([128, 512], F32, tag="o", name=f"obt{t}")
                      for t in range(4)]
                for j in range(NJ):
                    vj = vq_tiles[j]
                    for ib in range(2, 4):
                        eTb = eT(j, ib * 128, (ib + 1) * 128)
                        nc.tensor.matmul(
                            prB[ib - 2][:], eTb, ones_h[:],
                            start=(j == 0), stop=(j == NJ - 1),
                        )
                        for dq in range(2):
                            nc.tensor.matmul(
                                ob[(ib - 2) * 2 + dq][:], eTb,
                                vj[:, dq * 512:(dq + 1) * 512],
                                start=(j == 0), stop=(j == NJ - 1),
                            )
                for ib in range(2, 4):
                    nc.vector.tensor_copy(rsum[:, ib:ib + 1],
                                          prB[ib - 2][:, 0:1])
                nc.vector.reciprocal(rinv[:, 2:4], rsum[:, 2:4])
                for ib in range(2, 4):
                    for dq in range(2):
                        scale_store(ob[(ib - 2) * 2 + dq], ib, dq * 512,
                                    f"B{ib}_{dq}")

                # ---- six passes: O[all ib, one 512-col block], 1024:4096
                for p in range(6):
                    c0 = 1024 + p * 512
                    oc = [psO.tile([128, 512], F32, tag="o",
                                   name=f"oc{p}_{t}") for t in range(NI)]
                    for j in range(NJ):
                        vj = vpool.tile([128, 512], BF16, tag="v",
                                        name=f"v{p}_{j}")
                        nc.sync.dma_start(
                            out=vj[:],
                            in_=v_dram[j * 128:(j + 1) * 128, c0:c0 + 512],
                        )
                        for ib in range(NI):
                            nc.tensor.matmul(
                                oc[ib][:], eT(j, ib * 128, (ib + 1) * 128),
                                vj[:],
                                start=(j == 0), stop=(j == NJ - 1),
                            )
                    for ib in range(NI):
                        scale_store(oc[ib], ib, c0, f"C{p}_{ib}")

    nc.compile()
    return nc


_NC_CACHE = None


def _get_nc():
    global _NC_CACHE
    if _NC_CACHE is None:
        _NC_CACHE = _build_nc()
    return _NC_CACHE


def _make_in_maps(x: np.ndarray) -> list:
    x = np.asarray(x)
    n, c, h, w = x.shape
    assert (n, c, h * w) == (N, 3, D), f"unexpected shape {x.shape}"
    xr = np.ascontiguousarray(x.reshape(n, c, h * w).transpose(1, 0, 2))
    q_full, k, v = xr[0], xr[1], xr[2]
    # kT[jb, p, db, jj] = k[jb*128+jj, db*128+p] -- per-(jb) contiguous 2MB
    kT = np.ascontiguousarray(
        k.reshape(NJ, 128, ND, 128).transpose(0, 3, 2, 1)
    )
    v_bf16 = v.astype(ml_dtypes.bfloat16)
    in_maps = []
    for core in range(N_CORES):
        qc = q_full[core * M:(core + 1) * M]          # [M, D]
        # qT[p, db, i] = q[i, db*128+p]
        qTc = np.ascontiguousarray(
            qc.reshape(M, ND, 128).transpose(2, 1, 0)
        )
        in_maps.append({"qT": qTc, "kT": kT, "v": v_bf16})
    return in_maps


def kernel(x: np.ndarray) -> np.ndarray:
    nc = _get_nc()
    res = run_bass_kernel_spmd(nc, _make_in_maps(x), core_ids=list(range(N_CORES)))
    out = np.concatenate([r["o"] for r in res.results], axis=0)
    return out.astype(np.float32)


# revision 24
# speedup vs baseline: 1.1721x; 1.0171x over previous
"""TRN2 Bass kernel for nn_Attention_86260123173325.

Single-head attention over N=4096 tokens, feature dim HW=4096:
  q, k, v = x[:,0], x[:,1], x[:,2] reshaped to [4096, 4096]
  out = softmax(0.5 * q @ k.T) @ v

Sharding: q rows split across 8 cores (512 rows each); k, v replicated.
Host-side marshaling pre-transposes q and k into PE-ready contraction-major
layouts (the PE reduces along the partition dim), and converts v to bf16
(phase-2 value quantization contributes <1e-3 output error while halving the
v HBM stream, which paces phase 2 otherwise).

Per-core algorithm (phase-1 matmuls in f32r = TF32-like; phase 2 in bf16):
  - Phase 1, per 128-row k block j: R^T[j,:] = k_j @ q^T via 32 accumulated
    f32r matmuls. Keep R^T in SBUF (fp32), and accumulate a row statistic
    W_i = sum_j exp(0.1*R_ij - 40) via exp on ACT + ones^T @ E_t matmuls.
    The -40 bias keeps W far below ~2^64 where the HW exp/f32r/ln chain was
    observed to break.  kT blocks stream as two 1MB DMAs on the two HWDGE
    rings (sync+scalar) to hold >280GB/s; block 0 is split in four so the
    first matmul can start ~5us in.
  - shift_i = 5*(ln(W_i) + 40) >= rowmax_i; any per-row shift cancels in the
    final normalization, so exp(dp - shift) is an exact softmax numerator.
  - Bridge: a few throwaway matmuls keep the PE busy through the ln->shift
    serial chain so the HAM clock gate never re-throttles (a >3.4us PE idle
    gap costs ~35us of half-clock execution afterwards).
  - Pass 2 (bf16): eT = exp(0.5*R - shift), emitted in 256-wide half tiles
    so the first phase-2 matmul trails phase 1 by ~2us.
  - Phase 2: O = (E @ v) * (1/rowsum), j-outer so each eT weight load feeds
    2 column-block matmuls, in five passes that fit the 8 PSUM banks:
      A: rowsum(ib0,ib1) + O[ib0/1, cols 0:1024]   (races the exp pass)
      B: rowsum(ib2,ib3) + O[ib2/3, cols 0:1024]   (v tiles reused from A)
      then six passes O[all ib, one 512-col block each] for cols 1024:4096
      (4 of 6 shared PSUM banks active, 2 spare so passes overlap).
    Rowsums ride the same weight loads; reciprocals are computed per ib-pair
    so pass-A PSUM banks release before pass B needs them.
"""
import sys

sys.path.insert(0, "/opt/trn_rl_repo")

import ml_dtypes
import numpy as np

import concourse.bass as bass
import concourse.tile as tile
from concourse import bacc, mybir
from concourse.bass_utils import run_bass_kernel_spmd

F32 = mybir.dt.float32
F32R = mybir.dt.float32r
BF16 = mybir.dt.bfloat16
EXP = mybir.ActivationFunctionType.Exp
LN = mybir.ActivationFunctionType.Ln

N_CORES = 8
N = 4096          # tokens (keys)
D = 4096          # feature dim (H*W)
M = N // N_CORES  # q rows per core = 512
NJ = N // 128     # 32 key blocks
ND = D // 128     # 32 feature blocks
NI = M // 128     # 4 q-row blocks per core
T_STAT = 0.2      # stage-1 temperature: exp(t*dp - 40) = exp(0.1*R - 40)
STAT_BIAS = 40.0
N_BRIDGE = 10     # PE keep-warm matmuls across the softmax serial chain


def _build_nc():
    nc = bacc.Bacc(None, target_bir_lowering=False, debug=False)

    # qT[p, db, i] = q[i, db*128+p]; kT[jb, p, db, jj] = k[jb*128+jj, db*128+p]
    qT_dram = nc.dram_tensor("qT", [128, ND, M], F32R, kind="ExternalInput")
    kT_dram = nc.dram_tensor("kT", [NJ, 128, ND, 128], F32R, kind="ExternalInput")
    v_dram = nc.dram_tensor("v", [N, D], BF16, kind="ExternalInput")
    o_dram = nc.dram_tensor("o", [M, D], F32, kind="ExternalOutput")

    with tile.TileContext(nc) as tc:
        with tc.tile_pool(name="persist", bufs=1) as persist:
            # R^T storage, [j-within-block, j-block, i] (fp32, exact scores)
            s_sb = persist.tile([128, NJ, M], F32)

            ones_f = persist.tile([128, 128], F32, tag="ones_f")
            nc.vector.memset(ones_f[:], 1.0)
            # all-ones f32r [128,128]: W-stat lhsT (output lands broadcast on
            # all 128 partitions) + bridge matmuls
            ones_r = persist.tile([128, 128], F32R, tag="ones_r")
            nc.vector.tensor_copy(ones_r[:], ones_f[:])
            # bf16 ones [128,2]: phase-2 rowsum rhs
            ones_h = persist.tile([128, 2], BF16, tag="ones_h")
            nc.vector.tensor_copy(ones_h[:], ones_f[:, 0:2])

            zero_b = persist.tile([128, 1], F32, tag="zero_b")
            nc.vector.memset(zero_b[:], 0.0)

            # stage-1 exp bias: keeps W = sum exp(0.2*dp - 40) well under
            # ~2^64, where the HW exp/f32r-matmul/ln chain breaks
            stat_b = persist.tile([128, 1], F32, tag="stat_b")
            nc.vector.memset(stat_b[:], -STAT_BIAS)

            # exp bias for pass 2: exp(0.5*(R - 10*lnW - 400))
            #                    = exp(0.5*R - 5*lnW - 200)
            e2_b = persist.tile([128, 1], F32, tag="e2_b")
            nc.vector.memset(e2_b[:], -0.5 * STAT_BIAS * 2.0 / T_STAT)
            w_ln = persist.tile([128, M], F32, tag="w_ln")
            rsum = persist.tile([128, NI], F32, tag="rsum")
            rinv = persist.tile([128, NI], F32, tag="rinv")
            br_sb = persist.tile([128, 2], F32, tag="br_sb")

            # ---------------- phase 1: R^T blocks + W stats ----------------
            with (
                tc.tile_pool(name="qT", bufs=1) as qTpool,
                tc.tile_pool(name="kT", bufs=3) as kTpool,
                tc.tile_pool(name="ett", bufs=2) as etpool,
                tc.tile_pool(name="psS", bufs=2, space="PSUM") as psS,
                tc.tile_pool(name="psW", bufs=1, space="PSUM") as psWp,
                tc.tile_pool(name="psBr", bufs=1, space="PSUM") as psBr,
            ):
                # qT in 16 half-chunk tiles (0.5MB each) -> fine-grained
                # deps.  DMA emission is need-order interleaved with the
                # first kT blocks across both HWDGE rings so block 0's
                # matmul dblk=2b starts about when chunk b lands.
                NQ = 16
                qT_parts = [
                    qTpool.tile([128, ND // NQ, M], F32R, tag=f"qT{b}",
                                name=f"qT{b}")
                    for b in range(NQ)
                ]

                def qT_dma(b, eng):
                    eng.dma_start(
                        out=qT_parts[b][:],
                        in_=qT_dram[:, b * (ND // NQ):(b + 1) * (ND // NQ), :],
                    )

                def qT_slice(dblk):
                    return qT_parts[dblk // (ND // NQ)][:, dblk % (ND // NQ), :]

                kT_tiles = {}

                def kT_dma(j, part, eng, nsl=2):
                    if j not in kT_tiles:
                        kT_tiles[j] = kTpool.tile([128, ND, 128], F32R,
                                                  tag="kT", name=f"kT{j}")
                    kt = kT_tiles[j]
                    step = ND // nsl
                    eng.dma_start(
                        out=kt[:, part * step:(part + 1) * step, :],
                        in_=kT_dram[j][:, part * step:(part + 1) * step, :],
                    )

                kT_dma(0, 0, nc.scalar, nsl=4)
                qT_dma(0, nc.scalar)
                kT_dma(0, 1, nc.sync, nsl=4)
                qT_dma(1, nc.sync)
                kT_dma(0, 2, nc.scalar, nsl=4)
                qT_dma(2, nc.scalar)
                kT_dma(0, 3, nc.sync, nsl=4)
                qT_dma(3, nc.sync)
                for b in range(4, NQ):
                    qT_dma(b, nc.scalar if b % 2 == 0 else nc.sync)
                    if b == 10:
                        kT_dma(1, 0, nc.sync)
                    elif b == 11:
                        kT_dma(1, 1, nc.scalar)

                psW = psWp.tile([128, M], F32)
                # wacc[jj, i] = sum_{j-blocks} exp(0.1*R - 40); the
                # cross-partition reduction to W happens in one matmul
                wacc = persist.tile([128, M], F32, tag="wacc")
                ett31 = None
                for j in range(NJ):
                    if j > 0 and j not in kT_tiles:
                        kT_dma(j, 0, nc.sync)
                        kT_dma(j, 1, nc.scalar)
                    kT = kT_tiles[j]
                    ps = psS.tile([128, M], F32, tag="S", name=f"ps{j}")
                    for dblk in range(ND):
                        nc.tensor.matmul(
                            ps[:],
                            kT[:, dblk, :],
                            qT_slice(dblk),
                            start=(dblk == 0),
                            stop=(dblk == ND - 1),
                        )
                    # stash raw scores R^T (fp32)
                    nc.vector.tensor_copy(s_sb[:, j, :], ps[:])
                    # W stat: exp(0.1*R - 40), accumulated on the DVE
                    ett = etpool.tile([128, M], F32R, tag="ett", name=f"et{j}")
                    nc.scalar.activation(
                        out=ett[:], in_=ps[:], func=EXP,
                        bias=stat_b[:], scale=0.5 * T_STAT,
                    )
                    if j == 0:
                        nc.gpsimd.tensor_copy(wacc[:], ett[:])
                    else:
                        nc.gpsimd.tensor_add(wacc[:], wacc[:], ett[:])
                    if j == NJ - 2:
                        ett_pin = ett

                # PE bridge across the wacc/ln/shift serial chain: gated on
                # ett30 so it starts the moment the last score matmul ends,
                # overwritten in place so it serializes back-to-back on the
                # PE.  Split around the psW reduction (which waits on the
                # gpsimd wacc chain tail) so no PE-idle window exceeds the
                # ~3.4us HAM re-throttle threshold.
                br = psBr.tile([128, M], F32)
                for r in range(N_BRIDGE):
                    nc.tensor.matmul(
                        br[:], ones_r[:], ett_pin[:],
                        start=True, stop=True, skip_group_check=True,
                    )

                # cross-partition reduce: psW[p, i] = W_i (broadcast), f32
                nc.tensor.matmul(psW[:], ones_f[:], wacc[:],
                                 start=True, stop=True)

                for r in range(6):
                    nc.tensor.matmul(
                        br[:], ones_r[:], ett_pin[:],
                        start=True, stop=True, skip_group_check=True,
                    )
                nc.vector.tensor_copy(br_sb[:], br[:, 0:2])

                # lnW' (psW rows are identical so this lands broadcast);
                # the 10*lnW+400 shift is fused into the pass-2
                # scalar_tensor_tensor + exp bias.  Half tiles shorten the
                # serial chain ahead of the first phase-2 consumer.
                for h in range(2):
                    hs = slice(h * (M // 2), (h + 1) * (M // 2))
                    nc.scalar.activation(
                        out=w_ln[:, hs], in_=psW[:, hs], func=LN,
                        bias=zero_b[:], scale=1.0,
                    )

            # ---------------- phase 2: eT = exp(0.5*R - shift); O ----------
            with (
                tc.tile_pool(name="eTp", bufs=1) as eTpool,
                tc.tile_pool(name="tmp", bufs=4) as tmpool,
                tc.tile_pool(name="vq0", bufs=1) as vq0pool,
                tc.tile_pool(name="vrot", bufs=10) as vpool,
                tc.tile_pool(name="osb", bufs=8) as opool,
                tc.tile_pool(name="psO", bufs=6, space="PSUM") as psO,
                tc.tile_pool(name="psR", bufs=2, space="PSUM") as psR,
            ):
                eT_t = eTpool.tile([128, NJ, M], BF16, name="eT_t")

                def eT(j, i0, i1):
                    return eT_t[:, j, i0:i1]

                COPY = mybir.ActivationFunctionType.Copy

                def scale_store(ps_tile, ib, c0, label, on_act=False):
                    # alternating DVE / ACT halves the serial psum-release
                    # latency at pass boundaries
                    osb = opool.tile([128, 512], F32, tag="osb",
                                     name=f"ob_{label}")
                    if on_act:
                        nc.scalar.activation(
                            out=osb[:], in_=ps_tile[:], func=COPY,
                            bias=0.0, scale=rinv[:, ib:ib + 1],
                        )
                    else:
                        nc.vector.tensor_scalar_mul(
                            osb[:], ps_tile[:], rinv[:, ib:ib + 1]
                        )
                    nc.scalar.dma_start(
                        out=o_dram[ib * 128:(ib + 1) * 128, c0:c0 + 512],
                        in_=osb[:],
                    )

                def exp_block(j):
                    # tmp = s_sb - 10*lnW; exp applies *0.5 and -200 bias.
                    # First two blocks in halves so the first phase-2 matmul
                    # trails the ln by ~1.5us; the rest full-width (fewer
                    # DVE/ACT dispatches).
                    nh = 2 if j < 2 else 1
                    for h in range(nh):
                        hs = slice(h * (M // nh), (h + 1) * (M // nh))
                        tmp = tmpool.tile([128, M // nh], F32, tag="tmp",
                                          name=f"tmp{j}_{h}")
                        nc.vector.scalar_tensor_tensor(
                            tmp[:], w_ln[:, hs], -2.0 / T_STAT,
                            s_sb[:, j, hs],
                            mybir.AluOpType.mult, mybir.AluOpType.add,
                        )
                        nc.scalar.activation(
                            out=eT_t[:, j, hs], in_=tmp[:],
                            func=EXP, bias=e2_b[:], scale=0.5,
                        )

                # rowsums: per-(j, ib) tiny matmuls ride the pass A/B weight
                # loads.  Separate PSUM banks per pass — sharing one bank is
                # a fatal PE-write/DVE-read bank conflict (pass B's matmuls
                # race pass A's rowsum copies).

                # ---- pass A: exp pass + rowsum(ib0/1) + O[ib0/1, 0:1024]
                prA = [psR.tile([128, 2], F32, tag="pr", name=f"prA{ib}")
                       for ib in range(2)]
                oa = [psO.tile([128, 512], F32, tag="o", name=f"oa{t}")
                      for t in range(4)]
                vq_tiles = []
                for j in range(NJ):
                    vj = vq0pool.tile([128, 1024], BF16, tag=f"vq{j}",
                                      name=f"vq{j}")
                    nc.sync.dma_start(
                        out=vj[:], in_=v_dram[j * 128:(j + 1) * 128, 0:1024]
                    )
                    vq_tiles.append(vj)
                    exp_block(j)
                    for ib in range(2):
                        eTb = eT(j, ib * 128, (ib + 1) * 128)
                        nc.tensor.matmul(
                            prA[ib][:], eTb, ones_h[:],
                            start=(j == 0), stop=(j == NJ - 1),
                        )
                        for dq in range(2):
                            nc.tensor.matmul(
                                oa[ib * 2 + dq][:], eTb,
                                vj[:, dq * 512:(dq + 1) * 512],
                                start=(j == 0), stop=(j == NJ - 1),
                            )
                for ib in range(2):
                    nc.vector.tensor_copy(rsum[:, ib:ib + 1],
                                          prA[ib][:, 0:1])
                nc.vector.reciprocal(rinv[:, 0:2], rsum[:, 0:2])
                for ib in range(2):
                    for dq in range(2):
                        scale_store(oa[ib * 2 + dq], ib, dq * 512,
                                    f"A{ib}_{dq}", on_act=(dq == 1))

                # ---- pass B: rowsum(ib2/3) + O[ib2/3, 0:1024], v reused
                prB = [psR.tile([128, 2], F32, tag="pr", name=f"prB{ib}")
                       for ib in range(2)]
                ob = [psO.tile([128, 512], F32, tag="o", name=f"obt{t}")
                      for t in range(4)]
                for j in range(NJ):
                    vj = vq_tiles[j]
                    for ib in range(2, 4):
                        eTb = eT(j, ib * 128, (ib + 1) * 128)
                        nc.tensor.matmul(
                            prB[ib - 2][:], eTb, ones_h[:],
                            start=(j == 0), stop=(j == NJ - 1),
                        )
                        for dq in range(2):
                            nc.tensor.matmul(
                                ob[(ib - 2) * 2 + dq][:], eTb,
                                vj[:, dq * 512:(dq + 1) * 512],
                                start=(j == 0), stop=(j == NJ - 1),
                            )
                for ib in range(2, 4):
                    nc.vector.tensor_copy(rsum[:, ib:ib + 1],
                                          prB[ib - 2][:, 0:1])
                nc.vector.reciprocal(rinv[:, 2:4], rsum[:, 2:4])
                for ib in range(2, 4):
                    for dq in range(2):
                        scale_store(ob[(ib - 2) * 2 + dq], ib, dq * 512,
                                    f"B{ib}_{dq}", on_act=(dq == 1))

                # ---- six passes: O[all ib, one 512-col block], 1024:4096
                for p in range(6):
                    c0 = 1024 + p * 512
                    oc = [psO.tile([128, 512], F32, tag="o",
                                   name=f"oc{p}_{t}") for t in range(NI)]
                    for j in range(NJ):
                        vj = vpool.tile([128, 512], BF16, tag="v",
                                        name=f"v{p}_{j}")
                        nc.sync.dma_start(
                            out=vj[:],
                            in_=v_dram[j * 128:(j + 1) * 128, c0:c0 + 512],
                        )
                        for ib in range(NI):
                            nc.tensor.matmul(
                                oc[ib][:], eT(j, ib * 128, (ib + 1) * 128),
                                vj[:],
                                start=(j == 0), stop=(j == NJ - 1),
                            )
                    for ib in range(NI):
                        scale_store(oc[ib], ib, c0, f"C{p}_{ib}",
                                    on_act=(ib % 2 == 1))

    nc.compile()
    return nc


_NC_CACHE = None


def _get_nc():
    global _NC_CACHE
    if _NC_CACHE is None:
        _NC_CACHE = _build_nc()
    return _NC_CACHE


def _make_in_maps(x: np.ndarray) -> list:
    x = np.asarray(x)
    n, c, h, w = x.shape
    assert (n, c, h * w) == (N, 3, D), f"unexpected shape {x.shape}"
    xr = np.ascontiguousarray(x.reshape(n, c, h * w).transpose(1, 0, 2))
    q_full, k, v = xr[0], xr[1], xr[2]
    # kT[jb, p, db, jj] = k[jb*128+jj, db*128+p] -- per-(jb) contiguous 2MB
    kT = np.ascontiguousarray(
        k.reshape(NJ, 128, ND, 128).transpose(0, 3, 2, 1)
    )
    v_bf16 = v.astype(ml_dtypes.bfloat16)
    in_maps = []
    for core in range(N_CORES):
        qc = q_full[core * M:(core + 1) * M]          # [M, D]
        # qT[p, db, i] = q[i, db*128+p]
        qTc = np.ascontiguousarray(
            qc.reshape(M, ND, 128).transpose(2, 1, 0)
        )
        in_maps.append({"qT": qTc, "kT": kT, "v": v_bf16})
    return in_maps


def kernel(x: np.ndarray) -> np.ndarray:
    nc = _get_nc()
    res = run_bass_kernel_spmd(nc, _make_in_maps(x), core_ids=list(range(N_CORES)))
    out = np.concatenate([r["o"] for r in res.results], axis=0)
    return out.astype(np.float32)
